# revision 43
# baseline (speedup 1.0000x reference)
"""Multi-head attention on 8 Trainium2 NeuronCores.

Problem: x[4, 2048, 1024], 16 heads x 64 dim.
  qkv = x @ w_qkv; attn = softmax(q k^T / 8); out = (attn v) @ w_out + b_out

Sharding: 8 cores = 4 batches x 2 head-groups (8 heads each).
Each core computes a partial out-projection over its 8 heads' dims;
host sums the two partials per batch and adds the bias.

Per-core pipeline (device matmuls in float32r = full-rate fp32):
  phase 1: qT,kT = (w_qk^T x^T) with d on partitions (x^T host-packed so
           each chunk load is one linear 2 MB DMA); v natural [n, 512]
           stored per head pair as [v_even | ones | v_odd] so the fused
           av matmul emits unnormalized out^T rows plus replicated softmax
           row-sums with no partition-axis reduction. Head-pair 0's
           attention is interleaved into the chunk loop (staircase) so
           ScalarE starts exp()s early.
  phase 2: per head pair: both heads' scoresT[j,i] in one PSUM tile
           (row-packed K=64 matmuls) -> one FD=1024 exp on ScalarE
           (scale=1/8 folded) -> fused av matmul -> DVE reciprocal+mul.
  phase 3: partial_out[n, 1024] = attnoutT^T @ w_out_rows, interleaved
           with the last pair's i-chunks.

Measured on the 8-core axon TRN2: ~400-525 us span across runs, median
~480 us (pipelined-slope method; the axon tunnel adds +-10% run noise),
cost-model schedule 392 us; output matches the fp32 reference to 1.76e-4
scale-relative max error (float32r matmul precision).
"""

import numpy as np

import concourse.bacc as bacc
import concourse.mybir as mybir
import concourse.tile as tile
from concourse.bass_utils import run_bass_kernel_spmd

F32 = mybir.dt.float32
F32R = mybir.dt.float32r
AF = mybir.ActivationFunctionType

B = 4          # batch
N = 2048       # sequence
DM = 1024      # model dim
NH = 16        # heads
DH = 64        # head dim
G = 2          # head groups (cores per batch)
HPC = NH // G  # heads per core = 8
CW = DH * HPC  # per-core qkv column width = 512

NCH = 256      # phase-1 x^T column chunk
ICH = 512      # phase-2 i (query) chunk (per head; a pair shares [128, 2*ICH])

KT = DM // 128      # 8 contraction tiles over d
MT = 2 * CW // 128  # 8 c-tiles for q|k
NJT = N // 128      # 16 j tiles
NIC = N // ICH      # 4 i chunks


def build_nc(reps=1):
    nc = bacc.Bacc(None, target_bir_lowering=False, debug=False)

    # xT is host-packed to [chunk, partition, k*NCH] so every phase-1 chunk
    # load is one fully linear 2 MB DMA
    xT = nc.declare_dram_parameter("xT", [N // NCH, 128, KT * NCH], F32R,
                                   isOutput=False)
    wqk = nc.declare_dram_parameter("wqk", [DM, 2 * CW], F32R, isOutput=False)
    wv = nc.declare_dram_parameter("wv", [DM, CW], F32R, isOutput=False)
    wo = nc.declare_dram_parameter("wo", [CW, DM], F32R, isOutput=False)
    out = nc.declare_dram_parameter("out", [N, DM], F32, isOutput=True)

    with tile.TileContext(nc) as tc:
        with (
            tc.tile_pool(name="cpool", bufs=1) as cpool,
            # 8 PSUM banks: "s" 2x[128,1024] scores/out-proj, "p1" 2x[128,512]
            # phase-1 projections, "av" 2x[128,512] attention accumulators
            tc.tile_pool(name="psA", bufs=2, space="PSUM") as psA,
            tc.tile_pool(name="psB", bufs=2, space="PSUM") as psB,
            tc.tile_pool(name="psC", bufs=2, space="PSUM") as psC,
            tc.tile_pool(name="epool", bufs=5) as epool,
        ):
          for _rep in range(reps):
            qkT_t = [cpool.tile([128, N], F32R, name=f"qkT{m}") for m in range(MT)]
            # v tile: per head pair [v_even | ones | v_odd] (3*64 cols) -> the
            # fused av+rowsum matmul takes a contiguous [128, 128] lhsT for
            # either head, sharing the ones block; for the odd head the
            # output rows come out as [sums | out] instead of [out | sums]
            v_t = [cpool.tile([128, (HPC // 2) * 3 * DH], F32R, name=f"v{j}")
                   for j in range(NJT)]

            def attn_cell(p, ic, jt, av2):
                """One (pair, i-chunk, j-tile) attention step."""
                qt, kt = qkT_t[p], qkT_t[MT // 2 + p]
                isl = slice(ic * ICH, (ic + 1) * ICH)
                s_ps = psA.tile([128, 2 * ICH], F32, name="s_ps", tag="s")
                for half in range(2):
                    off = half * DH
                    nc.tensor.matmul(
                        s_ps[:, half * ICH:(half + 1) * ICH],
                        kt[off:off + DH, jt * 128:(jt + 1) * 128],
                        qt[off:off + DH, isl],
                        start=True, stop=True,
                    )
                ex = epool.tile([128, 2 * ICH], F32R, name="ex", tag="ex")
                nc.scalar.activation(ex[:], s_ps[:], AF.Exp, scale=0.125)
                for half in range(2):
                    base = p * 3 * DH + half * DH
                    vl = v_t[jt][:, base:base + 2 * DH]
                    nc.tensor.matmul(
                        av2[half][:],
                        vl,
                        ex[:, half * ICH:(half + 1) * ICH],
                        start=(jt == 0), stop=(jt == NJT - 1),
                    )

            def normalize(p, ic, av2, aoT_t, lpool):
                isl = slice(ic * ICH, (ic + 1) * ICH)
                for half in range(2):
                    l = 2 * p + half
                    ct, coff = l // 2, (l % 2) * DH
                    # even head: rows [out | sums]; odd head: [sums | out]
                    o0, s0 = (0, DH) if half == 0 else (DH, 0)
                    rc = lpool.tile([DH, ICH], F32, name="rc", tag="rc", bufs=4)
                    nc.vector.reciprocal(rc[:], av2[half][s0:s0 + DH, :])
                    nc.vector.tensor_mul(
                        aoT_t[ct][coff:coff + DH, isl],
                        av2[half][o0:o0 + DH, :],
                        rc[:],
                    )

            # ---------------- phase 1 (+ pair-0/ic-0 staircase) ----------------
            # One early attention stream (pair 0, i-chunk 0) holds the two
            # "av" slots during phase 1 so ScalarE starts exp()s early.
            NST = 1  # staircase streams
            p0_av = [[psB.tile([128, ICH], F32, name=f"av_{ic}_{h}", tag="av")
                      for h in range(2)] for ic in range(NST)]
            p0_emitted = [0] * NIC  # next jt to emit per i-chunk stream

            with (
                tc.tile_pool(name="w1pool", bufs=1) as w1pool,
                tc.tile_pool(name="xpool", bufs=3) as xpool,
            ):
                wqk_t = [w1pool.tile([128, 2 * CW], F32R, name=f"wqk{k}")
                         for k in range(KT)]
                wv_t = [w1pool.tile([128, CW], F32R, name=f"wv{k}")
                        for k in range(KT)]

                for ch in range(N // NCH):
                    csl = slice(ch * NCH, (ch + 1) * NCH)
                    x_t = xpool.tile([128, KT * NCH], F32R, name="x_t", tag="x")
                    nc.sync.dma_start(x_t[:], xT[ch])
                    if ch == 0:  # weights after first x chunk is in flight
                        for k in range(KT):
                            nc.sync.dma_start(wqk_t[k][:],
                                              wqk[k * 128:(k + 1) * 128, :])
                        for k in range(KT):
                            nc.sync.dma_start(wv_t[k][:],
                                              wv[k * 128:(k + 1) * 128, :])
                    # qT / kT rows (c on partitions)
                    for m in range(MT):
                        pq = psC.tile([128, NCH], F32, name="pq", tag="p1")
                        for k in range(KT):
                            nc.tensor.matmul(
                                pq[:],
                                wqk_t[k][:, m * 128:(m + 1) * 128],
                                x_t[:, k * NCH:(k + 1) * NCH],
                                start=(k == 0), stop=(k == KT - 1),
                            )
                        nc.vector.tensor_copy(qkT_t[m][:, csl], pq[:])
                    # v natural (n on partitions), [v_l | ones] per head
                    for mt in range(NCH // 128):
                        j = ch * (NCH // 128) + mt
                        pv = psC.tile([128, CW], F32, name="pv", tag="p1")
                        for k in range(KT):
                            nc.tensor.matmul(
                                pv[:],
                                x_t[:, k * NCH + mt * 128:k * NCH + (mt + 1) * 128],
                                wv_t[k][:],
                                start=(k == 0), stop=(k == KT - 1),
                            )
                        v3 = v_t[j].rearrange("p (q c) -> p q c", c=3 * DH)
                        pv3 = pv[:].rearrange("p (l c) -> p l c", c=DH)
                        nc.vector.tensor_copy(v3[:, :, 0:DH], pv3[:, 0::2, :])
                        nc.vector.tensor_copy(v3[:, :, 2 * DH:3 * DH],
                                              pv3[:, 1::2, :])
                        nc.any.memset(v3[:, :, DH:2 * DH].bitcast(F32), 1.0)
                    # pair-0 staircase: emit every cell whose q-columns and
                    # j-tile inputs are now available
                    jt_avail = 2 * (ch + 1)        # v/k tiles ready
                    for ic in range(NST):
                        if (ic + 1) * ICH > (ch + 1) * NCH:
                            continue               # q columns not ready yet
                        while p0_emitted[ic] < min(jt_avail, NJT):
                            attn_cell(0, ic, p0_emitted[ic], p0_av[ic])
                            p0_emitted[ic] += 1

            # ---------------- phases 2 & 3 ----------------
            with tc.tile_pool(name="lpool", bufs=2) as lpool:
                aoT_t = [lpool.tile([128, N], F32R, name=f"aoT{c}", tag=f"aoT{c}",
                                    bufs=1) for c in range(CW // 128)]
                wo_t = [lpool.tile([128, DM], F32R, name=f"wo{c}", tag=f"wo{c}",
                                   bufs=1) for c in range(CW // 128)]
                for c in range(CW // 128):
                    nc.sync.dma_start(wo_t[c][:], wo[c * 128:(c + 1) * 128, :])

                def phase3_group(nt, h):
                    po = psC.tile([128, 512], F32, name="po", tag="p1")
                    for c in range(CW // 128):
                        nc.tensor.matmul(
                            po[:],
                            aoT_t[c][:, nt * 128:(nt + 1) * 128],
                            wo_t[c][:, h * 512:(h + 1) * 512],
                            start=(c == 0), stop=(c == CW // 128 - 1),
                        )
                    os_ = lpool.tile([128, 512], F32, name="os", tag="os")
                    nc.vector.tensor_copy(os_[:], po[:])
                    nc.sync.dma_start(
                        out[nt * 128:(nt + 1) * 128, h * 512:(h + 1) * 512],
                        os_[:],
                    )

                def phase3_chunk(ic):
                    for nt in range(ic * ICH // 128, (ic + 1) * ICH // 128):
                        for h in range(DM // 512):
                            phase3_group(nt, h)

                def alloc_av2(p, ic):
                    # alternate PSUM tags per i-chunk so consecutive chunks
                    # don't serialize on accumulator release (pair 3 keeps
                    # "av" — its "p1" slots serve the out-projection)
                    if p < HPC // 2 - 1 and ic % 2 == 1:
                        pool, tag = psC, "p1"
                    else:
                        pool, tag = psB, "av"
                    return [pool.tile([128, ICH], F32, name=f"av{h}", tag=tag)
                            for h in range(2)]

                # finish pair 0 (staircase stream + remaining i-chunks)
                for ic in range(NIC):
                    av2 = p0_av[ic] if ic < NST else alloc_av2(0, ic)
                    while p0_emitted[ic] < NJT:
                        attn_cell(0, ic, p0_emitted[ic], av2)
                        p0_emitted[ic] += 1
                    normalize(0, ic, av2, aoT_t, lpool)

                # pairs 1..3; pair 3 spreads the out-projection of the
                # previous i-chunk between its attention cells
                p3_groups = []  # pending (nt, h) out-proj groups
                for p in range(1, HPC // 2):
                    for ic in range(NIC):
                        av2 = alloc_av2(p, ic)
                        for jt in range(NJT):
                            attn_cell(p, ic, jt, av2)
                            if p == HPC // 2 - 1 and jt % 2 == 1 and p3_groups:
                                phase3_group(*p3_groups.pop(0))
                        normalize(p, ic, av2, aoT_t, lpool)
                        if p == HPC // 2 - 1:
                            p3_groups += [(nt, h)
                                          for nt in range(ic * ICH // 128,
                                                          (ic + 1) * ICH // 128)
                                          for h in range(DM // 512)]
                while p3_groups:
                    phase3_group(*p3_groups.pop(0))

    nc.finalize()
    return nc


_NC_CACHE = {}


def _get_nc():
    if "nc" not in _NC_CACHE:
        _NC_CACHE["nc"] = build_nc()
    return _NC_CACHE["nc"]


def kernel(x, w_qkv, w_out, b_out):
    x = np.ascontiguousarray(x, dtype=np.float32)
    w_qkv = np.asarray(w_qkv, dtype=np.float32)
    w_out = np.asarray(w_out, dtype=np.float32)
    b_out = np.asarray(b_out, dtype=np.float32)

    nc = _get_nc()
    in_maps = []
    xp_cache = {}
    for core in range(8):
        b, g = divmod(core, 2)
        if b not in xp_cache:
            # pack x[b]^T as [chunk, partition, k, n] so device chunk loads
            # are single linear DMAs
            xp_cache[b] = np.ascontiguousarray(
                x[b].T.reshape(KT, 128, N // NCH, NCH).transpose(2, 1, 0, 3)
            ).reshape(N // NCH, 128, KT * NCH)
        xTb = xp_cache[b]
        wq = w_qkv[:, g * CW:(g + 1) * CW]
        wk = w_qkv[:, DM + g * CW:DM + (g + 1) * CW]
        wv_ = w_qkv[:, 2 * DM + g * CW:2 * DM + (g + 1) * CW]
        in_maps.append({
            "xT": xTb,
            "wqk": np.ascontiguousarray(np.concatenate([wq, wk], axis=1)),
            "wv": np.ascontiguousarray(wv_),
            "wo": np.ascontiguousarray(w_out[g * CW:(g + 1) * CW, :]),
        })

    res = run_bass_kernel_spmd(nc, in_maps, core_ids=list(range(8)))
    _NC_CACHE["last_result"] = res
    out = np.empty((B, N, DM), np.float32)
    for b in range(B):
        out[b] = res.results[2 * b]["out"] + res.results[2 * b + 1]["out"] + b_out
    return out
